# revision 14
# baseline (speedup 1.0000x reference)
"""Llama attention block (RMSNorm + QKV + interleaved RoPE + causal GQA SDPA
+ o_proj + residual) on 8 Trainium2 NeuronCores, tensor-parallel over heads.

Per-core shard: 4 query heads + 1 KV head (H=32, HKV=8, 8 cores).
All device compute runs in "transposed" layout ([feature, seq]) so the whole
chain (projections -> RoPE -> scores -> PV -> o_proj) needs no large
transposes:
  - host passes x^T and pre-transposed weight shards
  - QKV projections produce Q^T/K^T/V^T directly (weights stationary)
  - RMSNorm rstd is computed on-device (sum of squares via ones-matmul) and
    folded into the RoPE cos/sin tables (and V scaling)
  - interleaved RoPE pair-swap is a 128x128 permutation matmul
  - scores are computed transposed [k, q]; softmax denominator via a
    ones-column matmul accumulated alongside the PV matmul
  - causal masking: blocks fully above the diagonal are skipped, diagonal
    blocks are masked with host-provided 0/1 tables
  - o_proj consumes O^T and produces the core's partial out^T, with the
    residual folded in as 0.125 * x^T per core (8 cores -> 1.0 * x)
Host gathers: sum of per-core partials, transpose back.
"""

import os
import sys

import numpy as np

for _p in ("/opt/trn_rl_repo", "/root/.axon_site/_ro/trn_rl_repo"):
    if os.path.isdir(_p) and _p not in sys.path:
        sys.path.insert(0, _p)

import concourse.bass as bass  # noqa: E402
import concourse.tile as tile  # noqa: E402
from concourse import bacc, mybir  # noqa: E402
from concourse.alu_op_type import AluOpType  # noqa: E402
from concourse.bass_utils import run_bass_kernel_spmd  # noqa: E402

# Problem shape (hardcoded per contract)
S, D = 2048, 4096
H, HKV, HD = 32, 8, 128
NCORES = 8
HQ = H // NCORES            # 4 query heads per core
OQ = HQ * HD                # 512: per-core q projection out dim
EPS = 1e-5
SCALE = 1.0 / float(np.sqrt(HD))

NSTRIP = 4                  # q/seq strips of 512
STRIP = 512
NDC = D // 128              # 32 contraction chunks
NKB = S // 128              # 16 key blocks

F32 = mybir.dt.float32
F32R = mybir.dt.float32r

TRACE = False               # set by test harness for profiling runs
_LAST = {}                  # test harness reads exec_time_ns etc. from here
_CACHE = {}


def build_program():
    nc = bacc.Bacc(
        "TRN2",
        target_bir_lowering=False,
        debug=False,
        enable_asserts=False,
        num_devices=NCORES,
    )

    xT = nc.declare_dram_parameter("xT", [D, S], F32R, isOutput=False).ap()
    wqT = nc.declare_dram_parameter("wqT", [D, OQ], F32R, isOutput=False).ap()
    wkT = nc.declare_dram_parameter("wkT", [D, HD], F32R, isOutput=False).ap()
    wvT = nc.declare_dram_parameter("wvT", [D, HD], F32R, isOutput=False).ap()
    woT = nc.declare_dram_parameter("woT", [OQ, D], F32R, isOutput=False).ap()
    cosT = nc.declare_dram_parameter("cosT", [HD, S], F32, isOutput=False).ap()
    sinTs = nc.declare_dram_parameter("sinTs", [HD, S], F32, isOutput=False).ap()
    masks = nc.declare_dram_parameter("masks", [128, 4, STRIP], F32, isOutput=False).ap()
    swp = nc.declare_dram_parameter("swp", [128, 128], F32R, isOutput=False).ap()
    onesd = nc.declare_dram_parameter("ones", [128, 1], F32R, isOutput=False).ap()
    ident = nc.declare_dram_parameter("ident", [128, 128], F32, isOutput=False).ap()
    outT = nc.declare_dram_parameter("outT", [D, S], F32, isOutput=True).ap()

    with tile.TileContext(nc) as tc:
        from contextlib import ExitStack

        ctx = ExitStack()
        with ctx:
            # ---- persistent pools (whole kernel lifetime) ----
            persist = ctx.enter_context(tc.tile_pool(name="persist", bufs=1))
            qrotT = persist.tile([128, HQ, S], F32R, tag="qrotT")   # 32KB/part
            krotT = persist.tile([128, S], F32R, tag="krotT")       # 8KB/part
            vnat = persist.tile([128, NKB, HD], F32R, tag="vnat")   # 8KB/part
            o_sb = persist.tile([128, HQ, S], F32R, tag="o_sb")     # 32KB/part

            singles = ctx.enter_context(tc.tile_pool(name="singles", bufs=1))
            cos_sb = singles.tile([128, S], F32, tag="cos")
            sin_sb = singles.tile([128, S], F32, tag="sin")
            mask_sb = singles.tile([128, 4, STRIP], F32, tag="mask")
            swp_sb = singles.tile([128, 128], F32R, tag="swp")
            id_sb = singles.tile([128, 128], F32, tag="id")
            ones_sb = singles.tile([128, 1], F32R, tag="ones")
            eps_sb = singles.tile([1, 1], F32, tag="eps")
            wk_sb = singles.tile([128, NDC, HD], F32R, tag="wk")    # 16KB/part
            wv_sb = singles.tile([128, NDC, HD], F32R, tag="wv")    # 16KB/part

            nc.sync.dma_start(out=cos_sb, in_=cosT)
            nc.sync.dma_start(out=sin_sb, in_=sinTs)
            nc.sync.dma_start(out=mask_sb, in_=masks)
            nc.sync.dma_start(out=swp_sb, in_=swp)
            nc.sync.dma_start(out=id_sb, in_=ident)
            nc.sync.dma_start(out=wk_sb, in_=wkT.rearrange("(c p) o -> p c o", p=128))
            nc.sync.dma_start(out=wv_sb, in_=wvT.rearrange("(c p) o -> p c o", p=128))
            nc.sync.dma_start(out=ones_sb, in_=onesd)
            nc.vector.memset(eps_sb, EPS)

            # =============== Phase 1: QKV projections + RMSNorm + RoPE =====
            with (
                tc.tile_pool(name="p1_sb", bufs=3) as p1,
                tc.tile_pool(name="p1_sb1", bufs=1) as p1s,
                tc.tile_pool(name="p1_psum", bufs=6, space="PSUM") as pp_qkv,
                tc.tile_pool(name="p1_ssq", bufs=1, space="PSUM") as pp_ssq,
                tc.tile_pool(name="p1_tr", bufs=1, space="PSUM") as pp_tr,
            ):
                for i in range(NSTRIP):
                    sl = slice(i * STRIP, (i + 1) * STRIP)
                    pq = [pp_qkv.tile([128, STRIP], F32, tag="pqkv", name=f"pq{c}")
                          for c in range(HQ)]
                    pk = pp_qkv.tile([128, STRIP], F32, tag="pqkv")
                    pv = pp_qkv.tile([128, STRIP], F32, tag="pqkv")
                    pssq = pp_ssq.tile([1, STRIP], F32, tag="ssq")

                    for dc in range(NDC):
                        xt = p1.tile([128, STRIP], F32R, tag="xt")
                        nc.sync.dma_start(out=xt, in_=xT[dc * 128:(dc + 1) * 128, sl])
                        wqt = p1.tile([128, OQ], F32R, tag="wqt")
                        nc.sync.dma_start(out=wqt, in_=wqT[dc * 128:(dc + 1) * 128, :])

                        st, sp = dc == 0, dc == NDC - 1
                        for c in range(HQ):
                            nc.tensor.matmul(
                                out=pq[c], lhsT=(wqt[:, c * 128:(c + 1) * 128]),
                                rhs=(xt), start=st, stop=sp)
                        nc.tensor.matmul(out=pk, lhsT=(wk_sb[:, dc, :]), rhs=(xt),
                                         start=st, stop=sp)
                        nc.tensor.matmul(out=pv, lhsT=(wv_sb[:, dc, :]), rhs=(xt),
                                         start=st, stop=sp)
                        xsq = p1.tile([128, STRIP], F32R, tag="xsq")
                        nc.vector.tensor_mul(xsq, xt, xt)
                        nc.tensor.matmul(out=pssq, lhsT=(ones_sb), rhs=(xsq),
                                         start=st, stop=sp)

                    # rstd row for this strip; fold into rope tables
                    rs = p1s.tile([1, STRIP], F32, tag="rs")
                    nc.scalar.activation(out=rs, in_=pssq,
                                         func=mybir.ActivationFunctionType.Sqrt,
                                         bias=eps_sb, scale=1.0 / D)
                    rr = p1s.tile([1, STRIP], F32, tag="rr")
                    nc.vector.reciprocal(rr, rs)
                    rrb = p1s.tile([128, STRIP], F32, tag="rrb")
                    nc.gpsimd.partition_broadcast(rrb, rr)
                    ch = p1s.tile([128, STRIP], F32, tag="ch")
                    sh = p1s.tile([128, STRIP], F32, tag="sh")
                    nc.vector.tensor_mul(ch, cos_sb[:, sl], rrb)
                    nc.vector.tensor_mul(sh, sin_sb[:, sl], rrb)

                    # evacuate + RoPE for the 4 q heads and k
                    for c in range(HQ + 1):
                        src = pq[c] if c < HQ else pk
                        dst = qrotT[:, c, sl] if c < HQ else krotT[:, sl]
                        raw = p1.tile([128, STRIP], F32R, tag="raw")
                        nc.vector.tensor_copy(raw, src)
                        psw = pp_qkv.tile([128, STRIP], F32, tag="pqkv")
                        nc.tensor.matmul(out=psw, lhsT=(swp_sb), rhs=(raw),
                                         start=True, stop=True)
                        m1 = p1.tile([128, STRIP], F32, tag="m1")
                        nc.vector.tensor_mul(m1, raw, ch)
                        m2 = p1.tile([128, STRIP], F32, tag="m2")
                        nc.vector.tensor_mul(m2, psw, sh)
                        nc.vector.tensor_add(dst, m1, m2)

                    # V: scale by rstd, then transpose 128x128 blocks
                    vt = p1s.tile([128, STRIP], F32, tag="vt")
                    nc.vector.tensor_mul(vt, pv, rrb)
                    for rblk in range(STRIP // 128):
                        j = i * 4 + rblk
                        ptr = pp_tr.tile([128, 128], F32, tag="ptr")
                        nc.tensor.transpose(
                            out=ptr, in_=vt[:, rblk * 128:(rblk + 1) * 128],
                            identity=id_sb)
                        nc.vector.tensor_copy(vnat[:, j, :], ptr)

            # =============== Phase 2: causal GQA attention =================
            with (
                tc.tile_pool(name="p2_sb", bufs=4) as p2,
                tc.tile_pool(name="p2_sb2", bufs=2) as p2b,
                tc.tile_pool(name="p2_ps", bufs=2, space="PSUM") as pp_s,
                tc.tile_pool(name="p2_po", bufs=2, space="PSUM") as pp_o,
                tc.tile_pool(name="p2_pd", bufs=2, space="PSUM") as pp_d,
            ):
                for h in range(HQ):
                    for i in range(NSTRIP):
                        sl = slice(i * STRIP, (i + 1) * STRIP)
                        njb = 4 * (i + 1)
                        po = pp_o.tile([128, STRIP], F32, tag="po")
                        pd = pp_d.tile([1, STRIP], F32, tag="pd")
                        for j in range(njb):
                            ps = pp_s.tile([128, STRIP], F32, tag="ps")
                            nc.tensor.matmul(
                                out=ps, lhsT=(krotT[:, j * 128:(j + 1) * 128]),
                                rhs=(qrotT[:, h, sl]), start=True, stop=True)
                            pt = p2.tile([128, STRIP], F32R, tag="pt")
                            nc.scalar.activation(
                                out=pt, in_=ps,
                                func=mybir.ActivationFunctionType.Exp, scale=SCALE)
                            if j >= 4 * i:
                                nc.vector.tensor_mul(pt, pt, mask_sb[:, j - 4 * i, :])
                            nc.tensor.matmul(out=po, lhsT=(vnat[:, j, :]),
                                             rhs=(pt), start=j == 0, stop=j == njb - 1)
                            nc.tensor.matmul(out=pd, lhsT=(ones_sb), rhs=(pt),
                                             start=j == 0, stop=j == njb - 1)
                        dr = p2b.tile([1, STRIP], F32, tag="dr")
                        nc.vector.reciprocal(dr, pd)
                        drb = p2b.tile([128, STRIP], F32, tag="drb")
                        nc.gpsimd.partition_broadcast(drb, dr)
                        nc.vector.tensor_mul(o_sb[:, h, sl], po, drb)

            # =============== Phase 3: o_proj + residual ====================
            with (
                tc.tile_pool(name="p3_sb", bufs=3) as p3,
                tc.tile_pool(name="p3_ps", bufs=4, space="PSUM") as pp_out,
            ):
                woTr = woT.rearrange("(h p) m -> p h m", p=128)
                for m in range(NDC):
                    msl = slice(m * 128, (m + 1) * 128)
                    wot = p3.tile([128, HQ, 128], F32R, tag="wot")
                    nc.sync.dma_start(out=wot, in_=woTr[:, :, msl])
                    for i in range(NSTRIP):
                        sl = slice(i * STRIP, (i + 1) * STRIP)
                        pout = pp_out.tile([128, STRIP], F32, tag="pout")
                        for h in range(HQ):
                            nc.tensor.matmul(
                                out=pout, lhsT=(wot[:, h, :]),
                                rhs=(o_sb[:, h, sl]), start=h == 0, stop=h == HQ - 1)
                        xtr = p3.tile([128, STRIP], F32R, tag="xtr")
                        nc.sync.dma_start(out=xtr, in_=xT[msl, sl])
                        osb = p3.tile([128, STRIP], F32, tag="osb")
                        nc.vector.scalar_tensor_tensor(
                            out=osb, in0=xtr, scalar=1.0 / NCORES, in1=pout,
                            op0=AluOpType.mult, op1=AluOpType.add)
                        nc.sync.dma_start(out=outT[msl, sl], in_=osb)

    nc.compile()
    return nc


def shard_inputs(hidden_states, cos, sin, rms_weight, wq, wk, wv, wo, **_):
    x2d = np.asarray(hidden_states, np.float32).reshape(S, D)
    xT = np.ascontiguousarray(x2d.T)
    g = np.asarray(rms_weight, np.float32)[None, :]          # fold into weights
    cosT = np.ascontiguousarray(np.asarray(cos, np.float32).reshape(S, HD).T)
    sinT = np.ascontiguousarray(np.asarray(sin, np.float32).reshape(S, HD).T)
    sinTs = sinT.copy()
    sinTs[0::2, :] *= -1.0

    # diagonal-block masks: mask[r][k, q] = 1 if q >= k + 128*r
    kk = np.arange(128)[:, None]
    qq = np.arange(STRIP)[None, :]
    masks = np.stack([(qq >= kk + 128 * r).astype(np.float32) for r in range(4)],
                     axis=1)                                  # [128, 4, 512]
    swp = np.zeros((128, 128), np.float32)
    idx = np.arange(128)
    swp[idx, idx ^ 1] = 1.0
    ident = np.eye(128, dtype=np.float32)

    in_maps = []
    for c in range(NCORES):
        wq_c = np.asarray(wq, np.float32)[c * OQ:(c + 1) * OQ] * g
        wk_c = np.asarray(wk, np.float32)[c * HD:(c + 1) * HD] * g
        wv_c = np.asarray(wv, np.float32)[c * HD:(c + 1) * HD] * g
        wo_c = np.asarray(wo, np.float32)[:, c * OQ:(c + 1) * OQ]
        in_maps.append({
            "xT": xT,
            "wqT": np.ascontiguousarray(wq_c.T),
            "wkT": np.ascontiguousarray(wk_c.T),
            "wvT": np.ascontiguousarray(wv_c.T),
            "woT": np.ascontiguousarray(wo_c.T),
            "cosT": cosT,
            "sinTs": sinTs,
            "masks": masks,
            "swp": swp,
            "ones": np.ones((128, 1), np.float32),
            "ident": ident,
        })
    return in_maps


def kernel(**inputs):
    if "nc" not in _CACHE:
        _CACHE["nc"] = build_program()
    nc = _CACHE["nc"]
    in_maps = shard_inputs(**inputs)
    res = run_bass_kernel_spmd(nc, in_maps, list(range(NCORES)), trace=TRACE)
    _LAST["res"] = res
    acc = np.zeros((D, S), np.float64)
    for c in range(NCORES):
        acc += res.results[c]["outT"].astype(np.float64)
    out = acc.T.astype(np.float32).reshape(1, S, D)
    return out


# revision 16
# speedup vs baseline: 707.2794x; 707.2794x over previous
"""Llama attention block (RMSNorm + QKV + interleaved RoPE + causal GQA SDPA
+ o_proj + residual) on 8 Trainium2 NeuronCores, tensor-parallel over heads.

Per-core shard: 4 query heads + 1 KV head (H=32, HKV=8, 8 cores).
All device compute runs in "transposed" layout ([feature, seq]) so the whole
chain (projections -> RoPE -> scores -> PV -> o_proj) needs no large
transposes:
  - host passes x^T and pre-transposed weight shards
  - QKV projections produce Q^T/K^T/V^T directly (weights stationary)
  - RMSNorm rstd is computed on-device (sum of squares via ones-matmul) and
    folded into the RoPE cos/sin tables (and V scaling)
  - interleaved RoPE pair-swap is a 128x128 permutation matmul
  - scores are computed transposed [k, q]; softmax denominator via a
    ones-column matmul accumulated alongside the PV matmul
  - causal masking: blocks fully above the diagonal are skipped, diagonal
    blocks are masked with host-provided 0/1 tables
  - o_proj consumes O^T and produces the core's partial out^T, with the
    residual folded in as 0.125 * x^T per core (8 cores -> 1.0 * x)
Host gathers: sum of per-core partials, transpose back.

float32r everywhere on the PE: full-rate (1 cycle/row) fp32 matmuls; every
on-chip producer feeding the PE writes f32r so walrus's rounding check holds.
"""

import os
import sys

import numpy as np

for _p in ("/opt/trn_rl_repo", "/root/.axon_site/_ro/trn_rl_repo"):
    if os.path.isdir(_p) and _p not in sys.path:
        sys.path.insert(0, _p)

import concourse.bass as bass  # noqa: E402
import concourse.tile as tile  # noqa: E402
from concourse import bacc, mybir  # noqa: E402
from concourse.alu_op_type import AluOpType  # noqa: E402
from concourse.bass_utils import run_bass_kernel_spmd  # noqa: E402

# Problem shape (hardcoded per contract)
S, D = 2048, 4096
H, HKV, HD = 32, 8, 128
NCORES = 8
HQ = H // NCORES            # 4 query heads per core
OQ = HQ * HD                # 512: per-core q projection out dim
EPS = 1e-5
SCALE = 1.0 / float(np.sqrt(HD))

NSTRIP = 4                  # q/seq strips of 512
STRIP = 512
NDC = D // 128              # 32 contraction chunks
NKB = S // 128              # 16 key blocks

F32 = mybir.dt.float32
F32R = mybir.dt.float32r

_CACHE = {}


def build_program(repeat=1):
    nc = bacc.Bacc(
        "TRN2",
        target_bir_lowering=False,
        debug=False,
        enable_asserts=False,
        num_devices=NCORES,
    )

    xT = nc.declare_dram_parameter("xT", [D, S], F32R, isOutput=False).ap()
    wqT = nc.declare_dram_parameter("wqT", [D, OQ], F32R, isOutput=False).ap()
    wkT = nc.declare_dram_parameter("wkT", [D, HD], F32R, isOutput=False).ap()
    wvT = nc.declare_dram_parameter("wvT", [D, HD], F32R, isOutput=False).ap()
    woT = nc.declare_dram_parameter("woT", [OQ, D], F32R, isOutput=False).ap()
    cosT = nc.declare_dram_parameter("cosT", [HD, S], F32, isOutput=False).ap()
    sinTs = nc.declare_dram_parameter("sinTs", [HD, S], F32, isOutput=False).ap()
    masks = nc.declare_dram_parameter("masks", [128, 4, STRIP], F32,
                                      isOutput=False).ap()
    swp = nc.declare_dram_parameter("swp", [128, 128], F32R, isOutput=False).ap()
    onesd = nc.declare_dram_parameter("ones", [128, 1], F32R, isOutput=False).ap()
    ident = nc.declare_dram_parameter("ident", [128, 128], F32, isOutput=False).ap()
    outT = nc.declare_dram_parameter("outT", [D, S], F32, isOutput=True).ap()

    from contextlib import ExitStack

    with tile.TileContext(nc) as tc, ExitStack() as ctx:
        # ---- persistent pools (whole kernel lifetime) ----
        persist = ctx.enter_context(tc.tile_pool(name="persist", bufs=1))
        qrotT = persist.tile([128, HQ, S], F32R, tag="qrotT")   # 32KB/part
        krotT = persist.tile([128, S], F32R, tag="krotT")       # 8KB/part
        vnat = persist.tile([128, NKB, HD], F32R, tag="vnat")   # 8KB/part
        o_sb = persist.tile([128, HQ, S], F32R, tag="o_sb")     # 32KB/part

        singles = ctx.enter_context(tc.tile_pool(name="singles", bufs=1))
        cos_sb = singles.tile([128, S], F32, tag="cos")
        sin_sb = singles.tile([128, S], F32, tag="sin")
        mask_sb = singles.tile([128, 4, STRIP], F32, tag="mask")
        swp_sb = singles.tile([128, 128], F32R, tag="swp")
        id_sb = singles.tile([128, 128], F32, tag="id")
        ones_sb = singles.tile([128, 1], F32R, tag="ones")
        eps_sb = singles.tile([1, 1], F32, tag="eps")
        wk_sb = singles.tile([128, NDC, HD], F32R, tag="wk")    # 16KB/part
        wv_sb = singles.tile([128, NDC, HD], F32R, tag="wv")    # 16KB/part

        nc.sync.dma_start(out=cos_sb, in_=cosT)
        nc.sync.dma_start(out=sin_sb, in_=sinTs)
        nc.sync.dma_start(out=mask_sb, in_=masks)
        nc.sync.dma_start(out=swp_sb, in_=swp)
        nc.sync.dma_start(out=id_sb, in_=ident)
        nc.sync.dma_start(out=wk_sb, in_=wkT.rearrange("(c p) o -> p c o", p=128))
        nc.sync.dma_start(out=wv_sb, in_=wvT.rearrange("(c p) o -> p c o", p=128))
        nc.sync.dma_start(out=ones_sb, in_=onesd)
        nc.vector.memset(eps_sb, EPS)

        for _rep in range(repeat):
            # =========== Phase 1: QKV projections + RMSNorm + RoPE =========
            with (
                tc.tile_pool(name="p1_sb", bufs=3) as p1,
                tc.tile_pool(name="p1_sb1", bufs=1) as p1s,
                tc.tile_pool(name="p1_psum", bufs=6, space="PSUM") as pp_qkv,
                tc.tile_pool(name="p1_ssq", bufs=1, space="PSUM") as pp_ssq,
                tc.tile_pool(name="p1_tr", bufs=1, space="PSUM") as pp_tr,
            ):
                for i in range(NSTRIP):
                    sl = slice(i * STRIP, (i + 1) * STRIP)
                    pq = [pp_qkv.tile([128, STRIP], F32, tag="pqkv", name=f"pq{c}")
                          for c in range(HQ)]
                    pk = pp_qkv.tile([128, STRIP], F32, tag="pqkv")
                    pv = pp_qkv.tile([128, STRIP], F32, tag="pqkv")
                    pssq = pp_ssq.tile([1, STRIP], F32, tag="ssq")

                    for dc in range(NDC):
                        xt = p1.tile([128, STRIP], F32R, tag="xt")
                        nc.sync.dma_start(out=xt,
                                          in_=xT[dc * 128:(dc + 1) * 128, sl])
                        wqt = p1.tile([128, OQ], F32R, tag="wqt")
                        nc.sync.dma_start(out=wqt,
                                          in_=wqT[dc * 128:(dc + 1) * 128, :])

                        st, sp = dc == 0, dc == NDC - 1
                        for c in range(HQ):
                            nc.tensor.matmul(
                                out=pq[c], lhsT=wqt[:, c * 128:(c + 1) * 128],
                                rhs=xt, start=st, stop=sp)
                        nc.tensor.matmul(out=pk, lhsT=wk_sb[:, dc, :], rhs=xt,
                                         start=st, stop=sp)
                        nc.tensor.matmul(out=pv, lhsT=wv_sb[:, dc, :], rhs=xt,
                                         start=st, stop=sp)
                        xsq = p1.tile([128, STRIP], F32R, tag="xsq")
                        nc.vector.tensor_mul(xsq, xt, xt)
                        nc.tensor.matmul(out=pssq, lhsT=ones_sb, rhs=xsq,
                                         start=st, stop=sp)

                    # rstd row for this strip; fold into rope tables
                    rs = p1s.tile([1, STRIP], F32, tag="rs")
                    nc.scalar.activation(out=rs, in_=pssq,
                                         func=mybir.ActivationFunctionType.Sqrt,
                                         bias=eps_sb, scale=1.0 / D)
                    rr = p1s.tile([1, STRIP], F32, tag="rr")
                    nc.vector.reciprocal(rr, rs)
                    rrb = p1s.tile([128, STRIP], F32, tag="rrb")
                    nc.gpsimd.partition_broadcast(rrb, rr)
                    ch = p1s.tile([128, STRIP], F32, tag="ch")
                    sh = p1s.tile([128, STRIP], F32, tag="sh")
                    nc.vector.tensor_mul(ch, cos_sb[:, sl], rrb)
                    nc.vector.tensor_mul(sh, sin_sb[:, sl], rrb)

                    # evacuate + RoPE for the 4 q heads and k
                    for c in range(HQ + 1):
                        src = pq[c] if c < HQ else pk
                        dst = qrotT[:, c, sl] if c < HQ else krotT[:, sl]
                        raw = p1.tile([128, STRIP], F32R, tag="raw")
                        nc.vector.tensor_copy(raw, src)
                        psw = pp_qkv.tile([128, STRIP], F32, tag="pqkv")
                        nc.tensor.matmul(out=psw, lhsT=swp_sb, rhs=raw,
                                         start=True, stop=True)
                        m1 = p1.tile([128, STRIP], F32, tag="m1")
                        nc.vector.tensor_mul(m1, raw, ch)
                        m2 = p1.tile([128, STRIP], F32, tag="m2")
                        nc.vector.tensor_mul(m2, psw, sh)
                        nc.vector.tensor_add(dst, m1, m2)

                    # V: scale by rstd, then transpose 128x128 blocks
                    vt = p1s.tile([128, STRIP], F32, tag="vt")
                    nc.vector.tensor_mul(vt, pv, rrb)
                    for rblk in range(STRIP // 128):
                        j = i * 4 + rblk
                        ptr = pp_tr.tile([128, 128], F32, tag="ptr")
                        nc.tensor.transpose(
                            out=ptr, in_=vt[:, rblk * 128:(rblk + 1) * 128],
                            identity=id_sb)
                        nc.vector.tensor_copy(vnat[:, j, :], ptr)

            # =========== Phase 2: causal GQA attention (strip-major) =======
            with (
                tc.tile_pool(name="p2_sb", bufs=4) as p2,
                tc.tile_pool(name="p2_sb2", bufs=2) as p2b,
                tc.tile_pool(name="p2_ps", bufs=2, space="PSUM") as pp_s,
                tc.tile_pool(name="p2_po", bufs=2, space="PSUM") as pp_o,
                tc.tile_pool(name="p2_pd", bufs=2, space="PSUM") as pp_d,
            ):
                for i in range(NSTRIP):
                    sl = slice(i * STRIP, (i + 1) * STRIP)
                    njb = 4 * (i + 1)
                    for h in range(HQ):
                        po = pp_o.tile([128, STRIP], F32, tag="po")
                        pd = pp_d.tile([1, STRIP], F32, tag="pd")
                        for j in range(njb):
                            ps = pp_s.tile([128, STRIP], F32, tag="ps")
                            nc.tensor.matmul(
                                out=ps, lhsT=krotT[:, j * 128:(j + 1) * 128],
                                rhs=qrotT[:, h, sl], start=True, stop=True)
                            pt = p2.tile([128, STRIP], F32R, tag="pt")
                            nc.scalar.activation(
                                out=pt, in_=ps,
                                func=mybir.ActivationFunctionType.Exp,
                                scale=SCALE)
                            if j >= 4 * i:
                                nc.vector.tensor_mul(pt, pt,
                                                     mask_sb[:, j - 4 * i, :])
                            nc.tensor.matmul(out=po, lhsT=vnat[:, j, :], rhs=pt,
                                             start=j == 0, stop=j == njb - 1)
                            nc.tensor.matmul(out=pd, lhsT=ones_sb, rhs=pt,
                                             start=j == 0, stop=j == njb - 1)
                        dr = p2b.tile([1, STRIP], F32, tag="dr")
                        nc.vector.reciprocal(dr, pd)
                        drb = p2b.tile([128, STRIP], F32, tag="drb")
                        nc.gpsimd.partition_broadcast(drb, dr)
                        nc.vector.tensor_mul(o_sb[:, h, sl], po, drb)

            # =========== Phase 3: o_proj + residual ========================
            with (
                tc.tile_pool(name="p3_sb", bufs=3) as p3,
                tc.tile_pool(name="p3_ps", bufs=4, space="PSUM") as pp_out,
            ):
                woTr = woT.rearrange("(h p) m -> p h m", p=128)
                for m in range(NDC):
                    msl = slice(m * 128, (m + 1) * 128)
                    wot = p3.tile([128, HQ, 128], F32R, tag="wot")
                    nc.sync.dma_start(out=wot, in_=woTr[:, :, msl])
                    for i in range(NSTRIP):
                        sl = slice(i * STRIP, (i + 1) * STRIP)
                        pout = pp_out.tile([128, STRIP], F32, tag="pout")
                        for h in range(HQ):
                            nc.tensor.matmul(
                                out=pout, lhsT=wot[:, h, :],
                                rhs=o_sb[:, h, sl], start=h == 0,
                                stop=h == HQ - 1)
                        xtr = p3.tile([128, STRIP], F32R, tag="xtr")
                        nc.sync.dma_start(out=xtr, in_=xT[msl, sl])
                        osb = p3.tile([128, STRIP], F32, tag="osb")
                        nc.vector.scalar_tensor_tensor(
                            out=osb, in0=xtr, scalar=1.0 / NCORES, in1=pout,
                            op0=AluOpType.mult, op1=AluOpType.add)
                        nc.sync.dma_start(out=outT[msl, sl], in_=osb)

    nc.compile()
    return nc


def shard_inputs(hidden_states, cos, sin, rms_weight, wq, wk, wv, wo, **_):
    x2d = np.asarray(hidden_states, np.float32).reshape(S, D)
    xT = np.ascontiguousarray(x2d.T)
    g = np.asarray(rms_weight, np.float32)[None, :]          # fold into weights
    cosT = np.ascontiguousarray(np.asarray(cos, np.float32).reshape(S, HD).T)
    sinT = np.ascontiguousarray(np.asarray(sin, np.float32).reshape(S, HD).T)
    sinTs = sinT.copy()
    sinTs[0::2, :] *= -1.0

    # diagonal-block masks: mask[k, r, q] = 1 if q >= k + 128*r
    kk = np.arange(128)[:, None]
    qq = np.arange(STRIP)[None, :]
    masks = np.stack([(qq >= kk + 128 * r).astype(np.float32) for r in range(4)],
                     axis=1)                                  # [128, 4, 512]
    swp = np.zeros((128, 128), np.float32)
    idx = np.arange(128)
    swp[idx, idx ^ 1] = 1.0
    ident = np.eye(128, dtype=np.float32)

    in_maps = []
    for c in range(NCORES):
        wq_c = np.asarray(wq, np.float32)[c * OQ:(c + 1) * OQ] * g
        wk_c = np.asarray(wk, np.float32)[c * HD:(c + 1) * HD] * g
        wv_c = np.asarray(wv, np.float32)[c * HD:(c + 1) * HD] * g
        wo_c = np.asarray(wo, np.float32)[:, c * OQ:(c + 1) * OQ]
        in_maps.append({
            "xT": xT,
            "wqT": np.ascontiguousarray(wq_c.T),
            "wkT": np.ascontiguousarray(wk_c.T),
            "wvT": np.ascontiguousarray(wv_c.T),
            "woT": np.ascontiguousarray(wo_c.T),
            "cosT": cosT,
            "sinTs": sinTs,
            "masks": masks,
            "swp": swp,
            "ones": np.ones((128, 1), np.float32),
            "ident": ident,
        })
    return in_maps


def kernel(**inputs):
    if "nc" not in _CACHE:
        _CACHE["nc"] = build_program()
    nc = _CACHE["nc"]
    in_maps = shard_inputs(**inputs)
    res = run_bass_kernel_spmd(nc, in_maps, list(range(NCORES)))
    acc = np.zeros((D, S), np.float64)
    for c in range(NCORES):
        acc += res.results[c]["outT"].astype(np.float64)
    out = acc.T.astype(np.float32).reshape(1, S, D)
    return out
